# revision 8
# baseline (speedup 1.0000x reference)
"""MultiHeadAttention Trainium2 Bass kernel, 8-core SPMD — v3.

Problem: B=4, S=2048, E=2048, H=16, Dh=128; reshape-based (not transposed)
head split:  q = (x@Wq).reshape(B,H,S,Dh) etc., softmax over the QUERY axis,
out = attn.reshape(B,S,E).

Sharding: flattening (B,S) rows, row-block gp (128 rows) of x@W is exactly
head pair gp=(b,h).  Core c owns 8 consecutive pairs -> contiguous x rows
[1024c, 1024c+1024) and the same output rows.  No collectives.

v3 changes vs v2:
  - x pre-transposed on the host into the xtg SBUF layout -> phase_a
    (PE transposes + DVE copies) removed entirely.
  - Joint-max softmax: per-half reduce_max on DVE (HW forbids reading
    two PSUM inputs in one instruction, so no fused variant), combined
    with a tiny GpSimd MIN; the per-half flash correction (tiny Act
    exps, f/g stats, doubled vs scaling) is gone.  Math is the plain
    stable softmax.
  - Single vs tile per kj, scaled on the (otherwise idle) GpSimd engine;
    stat combines (joint max, L-sum) also on GpSimd.
  - attn output left in [d, (class, r)] layout; host un-transposes.
    PE out-transposes + their PSUM->SBUF copies removed.
  - Copies (q/k, yv, attn-acc) on DVE; Act does only the big exps.
"""

import numpy as np
from contextlib import ExitStack

import concourse.bass as bass
import concourse.tile as tile
from concourse import bacc, mybir
from concourse.bass import ds, ts
from concourse.bass_utils import run_bass_kernel_spmd

F32 = mybir.dt.float32
F32R = mybir.dt.float32r
BF16 = mybir.dt.bfloat16
AX = mybir.AxisListType.X
EXP = mybir.ActivationFunctionType.Exp
MAX = mybir.AluOpType.max
MIN = mybir.AluOpType.min

P = 128
NPAIR = 8          # (b,h) pairs per core
GRP = 4            # pairs per group (weights streamed once per group)
NGRP = NPAIR // GRP
NJ = 16            # 128-col blocks in E
NSEG = 8           # kj per attention accumulation segment
SCALE = 1.0 / np.sqrt(128.0)
FBIG = 3.0e38

_cache = {}


def _emit(nc, tc, ctx, xl, wq, wk, wv, out):
    sb = ctx.enter_context
    # SBUF pools
    pXT = sb(tc.tile_pool(name="pxt", bufs=1))       # XT group           32K
    pWQK = sb(tc.tile_pool(name="pwqk", bufs=3))     # w half-tiles       4K*3
    pWV = sb(tc.tile_pool(name="pwv", bufs=4))       # wv quarter-tiles   4K*4
    pQT = sb(tc.tile_pool(name="pqt", bufs=1))       # QT group           32K
    pKT = sb(tc.tile_pool(name="pkt", bufs=1))       # KT group           32K
    pYV = sb(tc.tile_pool(name="pyv", bufs=4))       # yv bf16 per pair   4K*4
    pSOFT = sb(tc.tile_pool(name="psoft", bufs=15))  # soft halves bf16   2K*15
    pVS = sb(tc.tile_pool(name="pvs", bufs=12))      # vs bf16 per kj     .25K*12
    pACC = sb(tc.tile_pool(name="pacc", bufs=2))     # attnT acc f32      8K*2
    pST = sb(tc.tile_pool(name="pst", bufs=12))      # small stats        tiny
    # PSUM pools: 3*2 + 2*1 = 8 banks
    psSC = sb(tc.tile_pool(name="pssc", bufs=3, space="PSUM"))   # [128,1024]
    psWK = sb(tc.tile_pool(name="pswk", bufs=2, space="PSUM"))   # [128,512]

    qt_tiles = {}
    kt_tiles = {}
    yv_tiles = {}

    def phase_x(grp):
        """DMA the group's pre-transposed x: [128(p), kb, pair, s] f32r."""
        xtg = pXT.tile([P, NJ, GRP, P], F32R, tag="xtg")
        for pi in range(GRP):
            nc.sync.dma_start(xtg[:, :, pi, :], xl[grp, pi])
        return xtg

    def phase_b_qk(grp, xtg):
        """Q/K projections, j-major over the group's 4 pairs.

        Output layout: qt/kt [128(d), pair, j, s] f32r kept in SBUF."""
        qtg = pQT.tile([P, GRP, NJ, P], F32R, tag="qtg")
        ktg = pKT.tile([P, GRP, NJ, P], F32R, tag="ktg")
        for j in range(NJ):
            for wd, dstg in ((wq, qtg), (wk, ktg)):
                ps = psWK.tile([P, 512], F32, tag="wk")
                for h in range(2):
                    wt = pWQK.tile([P, 8, P], F32R, tag="wqk")
                    nc.sync.dma_start(wt[:], wd[j, h])
                    for kb8 in range(8):
                        kb = h * 8 + kb8
                        nc.tensor.matmul(
                            ps[:], wt[:, kb8], xtg[:, kb],
                            start=(kb == 0), stop=(kb == NJ - 1),
                        )
                nc.vector.tensor_copy(
                    dstg[:, :, j, :], ps[:].rearrange("p (g s) -> p g s", g=GRP)
                )
        for pi in range(GRP):
            gp = grp * GRP + pi
            qt_tiles[gp] = qtg
            kt_tiles[gp] = ktg

    def phase_b_v(grp, xtg):
        """V projections: yv[pair] = [128(s), 2048(e)] bf16 in SBUF."""
        for pi in range(GRP):
            yv_tiles[grp * GRP + pi] = pYV.tile(
                [P, NJ * P], BF16, tag="yv", name=f"yv{grp * GRP + pi}"
            )
        for ec in range(8):
            wvts = []
            for q in range(4):
                wvt = pWV.tile([P, 4, 256], F32R, tag="wv")
                nc.sync.dma_start(wvt[:], wv[ec, q])
                wvts.append(wvt)
            for pi in range(GRP):
                gp = grp * GRP + pi
                ps = psWK.tile([P, 512], F32, tag="wk")
                for kb in range(NJ):
                    nc.tensor.matmul(
                        ps[:, :256], xtg[:, kb, pi], wvts[kb // 4][:, kb % 4],
                        start=(kb == 0), stop=(kb == NJ - 1),
                    )
                nc.vector.tensor_copy(yv_tiles[gp][:, ds(ec * 256, 256)], ps[:, :256])

    def phase_c(gp, pi):
        """Scores + joint-max softmax-over-q + attn, one pair.

        Emission order per pair: sc0..11, attn seg0 (kj0..7), sc12..15,
        attn seg1 (kj8..15) — the extra scores between a segment's last
        softmax chain and its attn matmuls keep PE busy during the
        DVE->Act->GpSimd chain latency."""
        qtg = qt_tiles.pop(gp)
        ktg = kt_tiles.pop(gp)
        yv = yv_tiles.pop(gp)
        acc = pACC.tile([P, NJ * P], F32, tag="acc")
        softs = {}
        vss = {}

        def sc(kj):
            kt_st = ktg[:, pi, kj, :]
            pss = []
            for h in range(2):
                ps = psSC.tile([P, 1024], F32, tag="sc")
                for c in range(2):
                    nc.tensor.matmul(
                        ps[:, ds(c * 512, 512)], kt_st,
                        qtg[:, pi, ds(h * 8 + c * 4, 4), :],
                        start=True, stop=True,
                    )
                pss.append(ps)
            # nm = -max over all 2048 q of this kj's k-rows: per-half
            # negated maxes on DVE, joint MIN combine on GpSimd.
            nm2 = pST.tile([P, 2], F32, tag="nm2")
            for h in range(2):
                nc.vector.reduce_max(
                    nm2[:, ds(h, 1)], pss[h][:], axis=AX, negate=True
                )
            nm = pST.tile([P, 1], F32, tag="nm")
            nc.vector.tensor_tensor(nm[:], nm2[:, :1], nm2[:, 1:], op=MIN)
            ls = pST.tile([P, 2], F32, tag="ls")
            for h in range(2):
                soft = pSOFT.tile([P, 1024], BF16, tag="soft")
                nc.scalar.activation(
                    soft[:], pss[h][:], EXP,
                    bias=nm[:], scale=1.0,
                    accum_out=ls[:, ds(h, 1)],
                )
                softs[kj, h] = soft
            lsum = pST.tile([P, 1], F32, tag="lsum")
            nc.gpsimd.tensor_add(lsum[:], ls[:, :1], ls[:, 1:])
            rcp = pST.tile([P, 1], F32, tag="rcp")
            nc.vector.reciprocal(rcp[:], lsum[:])
            vs = pVS.tile([P, P], BF16, tag="vs")
            nc.gpsimd.tensor_scalar_mul(vs[:], yv[:, ts(kj, P)], rcp[:])
            vss[kj] = vs

        def attn_seg(seg):
            for c in range(4):
                h = c // 2
                pa = psWK.tile([P, 512], F32, tag="wk")
                for i in range(NSEG):
                    kj = seg * NSEG + i
                    nc.tensor.matmul(
                        pa[:], vss[kj][:],
                        softs[kj, h][:, ds((c % 2) * 512, 512)],
                        start=(i == 0), stop=(i == NSEG - 1),
                    )
                if seg == 0:
                    nc.scalar.copy(acc[:, ds(c * 512, 512)], pa[:])
                else:
                    nc.vector.tensor_add(
                        acc[:, ds(c * 512, 512)], acc[:, ds(c * 512, 512)], pa[:]
                    )

        for kj in range(12):
            sc(kj)
        attn_seg(0)
        for kj in range(12, 16):
            sc(kj)
        attn_seg(1)
        softs.clear()
        vss.clear()
        nc.sync.dma_start(out[ds(gp * P, P), :], acc[:])

    for grp in range(NGRP):
        xtg = phase_x(grp)
        phase_b_qk(grp, xtg)
        phase_b_v(grp, xtg)
        for pi in range(GRP):
            phase_c(grp * GRP + pi, pi)


def build(compile=True):
    key = ("nc", compile)
    if key in _cache:
        return _cache[key]
    nc = bacc.Bacc("TRN2", target_bir_lowering=False, debug=False)
    # x pre-transposed on host: [grp, pair, p(e-chunk col), kb, s]
    xl = nc.dram_tensor("xl", [NGRP, GRP, P, NJ, P], F32R, kind="ExternalInput").ap()
    wq = nc.dram_tensor("wq", [NJ, 2, P, 8, P], F32R, kind="ExternalInput").ap()
    wk = nc.dram_tensor("wk", [NJ, 2, P, 8, P], F32R, kind="ExternalInput").ap()
    wv = nc.dram_tensor("wv", [8, 4, P, 4, 256], F32R, kind="ExternalInput").ap()
    # out rows = (gp, d), cols = (class, r); host un-transposes.
    out = nc.dram_tensor("out", [NPAIR * P, 2048], F32, kind="ExternalOutput").ap()
    with tile.TileContext(nc) as tc:
        with ExitStack() as ctx:
            _emit(nc, tc, ctx, xl, wq, wk, wv, out)
    if compile:
        nc.compile()
    _cache[key] = nc
    return nc


def _prep_inputs(x, w_query, w_key, w_value):
    x = np.ascontiguousarray(np.asarray(x, np.float32))
    wq = np.asarray(w_query, np.float32)
    wk = np.asarray(w_key, np.float32)
    wv = np.asarray(w_value, np.float32)
    B, S, E = x.shape
    xf = x.reshape(B * S, E)
    # [j, half, p, kb8, q] ; 1/sqrt(Dh) folded into wq
    wq_t = np.ascontiguousarray(
        (wq * SCALE).reshape(NJ, P, NJ, P).transpose(2, 1, 0, 3)
        .reshape(NJ, P, 2, 8, P).transpose(0, 2, 1, 3, 4)
    )
    wk_t = np.ascontiguousarray(
        wk.reshape(NJ, P, NJ, P).transpose(2, 1, 0, 3)
        .reshape(NJ, P, 2, 8, P).transpose(0, 2, 1, 3, 4)
    )
    wv_t = np.ascontiguousarray(
        wv.reshape(NJ, P, 8, 256).transpose(2, 1, 0, 3)
        .reshape(8, P, 4, 4, 256).transpose(0, 2, 1, 3, 4)
    )
    rows = NPAIR * P
    in_maps = []
    for c in range(8):
        xc = xf[c * rows:(c + 1) * rows]
        # xtg[g][pi][p, kb, s] = xc[(g*4+pi)*128 + s, kb*128 + p]
        xt = np.ascontiguousarray(
            xc.reshape(NGRP, GRP, P, NJ, P).transpose(0, 1, 4, 3, 2)
        )
        in_maps.append(dict(xl=xt, wq=wq_t, wk=wk_t, wv=wv_t))
    return in_maps, (B, S, E)


def kernel(x, w_query, w_key, w_value, _want_trace=False):
    in_maps, (B, S, E) = _prep_inputs(x, w_query, w_key, w_value)
    nc = build()
    res = run_bass_kernel_spmd(nc, in_maps, core_ids=list(range(8)),
                               trace=_want_trace)
    # per-core out rows=(gp,d), cols=(class c, r); un-transpose to
    # rows=(gp,r), cols=(c,d)
    blocks = []
    for r in res.results:
        rc = r["out"].reshape(NPAIR, P, NJ, P)
        blocks.append(rc.transpose(0, 3, 2, 1).reshape(NPAIR * P, 2048))
    outf = np.concatenate(blocks, axis=0)
    if _want_trace:
        kernel.last_result = res
    return outf.reshape(B, S, E)


# revision 19
# speedup vs baseline: 1.2089x; 1.2089x over previous
"""MultiHeadAttention Trainium2 Bass kernel, 8-core SPMD — v3.

Problem: B=4, S=2048, E=2048, H=16, Dh=128; reshape-based (not transposed)
head split:  q = (x@Wq).reshape(B,H,S,Dh) etc., softmax over the QUERY axis,
out = attn.reshape(B,S,E).

Sharding: flattening (B,S) rows, row-block gp (128 rows) of x@W is exactly
head pair gp=(b,h).  Core c owns 8 consecutive pairs -> contiguous x rows
[1024c, 1024c+1024) and the same output rows.  No collectives.

v3 changes vs v2:
  - x pre-transposed on the host into the xtg SBUF layout -> phase_a
    (PE transposes + DVE copies) removed entirely.
  - Per-half flash softmax (v2 chain: mm -> own max -> exp releases the
    PSUM tile after ~2.6us, so the 3-tile psSC rotation pipelines), but
    the cross-half correction factors f/g are computed in ONE batched
    [P,16] stage per 8-kj segment instead of per-kj small ops: removes
    v2's 128 tiny Act exps (108us) and most DVE stat spam.
  - GpSimd unused: measured 2.25us per [128,128] tensor_scalar (6x the
    cost model) — it poisons any chain it touches.
  - vs scaling split DVE/Act per segment; acc seg0-copies on Act,
    seg1-adds on DVE; Bv of the next group interleaved into phase_c.
  - attn output left in [d, (class, r)] layout; host un-transposes.
    PE out-transposes + their PSUM->SBUF copies removed.
  - Copies (q/k, yv, attn-acc) on DVE; Act does only the big exps.
"""

import numpy as np
from contextlib import ExitStack

import concourse.bass as bass
import concourse.tile as tile
from concourse import bacc, mybir
from concourse.bass import ds, ts
from concourse.bass_utils import run_bass_kernel_spmd

F32 = mybir.dt.float32
F32R = mybir.dt.float32r
BF16 = mybir.dt.bfloat16
AX = mybir.AxisListType.X
EXP = mybir.ActivationFunctionType.Exp
COPY = mybir.ActivationFunctionType.Copy
MAX = mybir.AluOpType.max
MIN = mybir.AluOpType.min

P = 128
NPAIR = 8          # (b,h) pairs per core
GRP = 4            # pairs per group (weights streamed once per group)
NGRP = NPAIR // GRP
NJ = 16            # 128-col blocks in E
NSEG = 8           # kj per attention accumulation segment
SCALE = 1.0 / np.sqrt(128.0)
FBIG = 3.0e38

_cache = {}


def _emit(nc, tc, ctx, xl, wq, wk, wv, out):
    sb = ctx.enter_context
    # SBUF pools
    pXT = sb(tc.tile_pool(name="pxt", bufs=1))       # XT group           32K
    pWQK = sb(tc.tile_pool(name="pwqk", bufs=2))     # w half-tiles       4K*2
    pWV = sb(tc.tile_pool(name="pwv", bufs=4))       # wv quarter-tiles   4K*4
    pQT = sb(tc.tile_pool(name="pqt", bufs=1))       # QT group           32K
    pKT = sb(tc.tile_pool(name="pkt", bufs=1))       # KT group           32K
    pYV = sb(tc.tile_pool(name="pyv", bufs=8))       # yv bf16 per pair   4K*8
    pSOFT = sb(tc.tile_pool(name="psoft", bufs=17))  # soft halves bf16   2K*17
    pVS = sb(tc.tile_pool(name="pvs", bufs=18))      # vs bf16            .25K*18
    pACC = sb(tc.tile_pool(name="pacc", bufs=1))     # attnT acc f32      8K
    pST = sb(tc.tile_pool(name="pst", bufs=14))      # small stats        tiny
    # PSUM pools: 3*2 + 2*1 = 8 banks
    psSC = sb(tc.tile_pool(name="pssc", bufs=3, space="PSUM"))   # [128,1024]
    psWK = sb(tc.tile_pool(name="pswk", bufs=2, space="PSUM"))   # [128,512]

    qt_tiles = {}
    kt_tiles = {}
    yv_tiles = {}

    def phase_x(grp):
        """DMA the group's pre-transposed x: [128(p), kb, pair, s] f32r."""
        xtg = pXT.tile([P, NJ, GRP, P], F32R, tag="xtg")
        for pi in range(GRP):
            nc.sync.dma_start(xtg[:, :, pi, :], xl[grp, pi])
        return xtg

    def phase_b_qk(grp, xtg):
        """Q/K projections, j-major over the group's 4 pairs.

        Output layout: qt/kt [128(d), pair, j, s] f32r kept in SBUF."""
        qtg = pQT.tile([P, GRP, NJ, P], F32R, tag="qtg")
        ktg = pKT.tile([P, GRP, NJ, P], F32R, tag="ktg")
        for j in range(NJ):
            for wd, dstg in ((wq, qtg), (wk, ktg)):
                ps = psWK.tile([P, 512], F32, tag="wk")
                for h in range(2):
                    wt = pWQK.tile([P, 8, P], F32R, tag="wqk")
                    nc.sync.dma_start(wt[:], wd[j, h])
                    for kb8 in range(8):
                        kb = h * 8 + kb8
                        nc.tensor.matmul(
                            ps[:], wt[:, kb8], xtg[:, kb],
                            start=(kb == 0), stop=(kb == NJ - 1),
                        )
                nc.vector.tensor_copy(
                    dstg[:, :, j, :], ps[:].rearrange("p (g s) -> p g s", g=GRP)
                )
        for pi in range(GRP):
            gp = grp * GRP + pi
            qt_tiles[gp] = qtg
            kt_tiles[gp] = ktg

    def phase_b_v_alloc(grp):
        for pi in range(GRP):
            yv_tiles[grp * GRP + pi] = pYV.tile(
                [P, NJ * P], BF16, tag="yv", name=f"yv{grp * GRP + pi}"
            )

    def phase_b_v_chunk(grp, xtg, ec, copies_on_act=False):
        """One 256-col chunk of the V projections for all 4 pairs."""
        wvts = []
        for q in range(4):
            wvt = pWV.tile([P, 4, 256], F32R, tag="wv")
            nc.sync.dma_start(wvt[:], wv[ec, q])
            wvts.append(wvt)
        for pi in range(GRP):
            gp = grp * GRP + pi
            ps = psWK.tile([P, 512], F32, tag="wk")
            for kb in range(NJ):
                nc.tensor.matmul(
                    ps[:, :256], xtg[:, kb, pi], wvts[kb // 4][:, kb % 4],
                    start=(kb == 0), stop=(kb == NJ - 1),
                )
            if copies_on_act:
                nc.scalar.copy(yv_tiles[gp][:, ds(ec * 256, 256)], ps[:, :256])
            else:
                nc.vector.tensor_copy(
                    yv_tiles[gp][:, ds(ec * 256, 256)], ps[:, :256]
                )

    def phase_b_v(grp, xtg):
        """V projections: yv[pair] = [128(s), 2048(e)] bf16 in SBUF."""
        phase_b_v_alloc(grp)
        for ec in range(8):
            phase_b_v_chunk(grp, xtg, ec)

    def phase_c(gp, pi, filler=None):
        """Scores + per-half flash softmax + batched correction + attn.

        sc(kj): per half, 2 matmuls -> own reduce_max (DVE, negated) ->
        exp with own-max bias (Act, releases the PSUM tile).  batch(seg)
        then computes the joint-max corrections for 8 kjs in [P,16]-wide
        ops and scales yv into per-(kj,half) vs tiles.
        `filler` (optional emission hook) injects PE work (next group's
        V projections) between pairs."""
        qtg = qt_tiles.pop(gp)
        ktg = kt_tiles.pop(gp)
        yv = yv_tiles.pop(gp)
        acc = pACC.tile([P, NJ * P], F32, tag="acc")
        softs = {}
        vss = {}
        stats = {}

        def sc(kj):
            seg, i = divmod(kj, NSEG)
            if i == 0:
                stats[seg] = (
                    pST.tile([P, 2 * NSEG], F32, tag="nmb", name=f"nmb{seg}"),
                    pST.tile([P, 2 * NSEG], F32, tag="lsb", name=f"lsb{seg}"),
                )
            nmb, lsb = stats[seg]
            kt_st = ktg[:, pi, kj, :]
            for h in range(2):
                ps = psSC.tile([P, 1024], F32, tag="sc")
                for c in range(2):
                    nc.tensor.matmul(
                        ps[:, ds(c * 512, 512)], kt_st,
                        qtg[:, pi, ds(h * 8 + c * 4, 4), :],
                        start=True, stop=True,
                    )
                sl = ds(i * 2 + h, 1)
                nc.vector.reduce_max(nmb[:, sl], ps[:], axis=AX, negate=True)
                soft = pSOFT.tile([P, 1024], BF16, tag="soft")
                nc.scalar.activation(
                    soft[:], ps[:], EXP,
                    bias=nmb[:, sl], scale=1.0,
                    accum_out=lsb[:, sl],
                )
                softs[kj, h] = soft

        def batch(seg, vs_act):
            """Joint-max correction for the segment's 8 kjs, [P,16]-wide.

            nmb holds -m_h per (kj,h); g_h = exp(m_h - m) / L with
            L = sum_h ls_h * exp(m_h - m)."""
            nmb, lsb = stats.pop(seg)
            n0 = nmb[:].rearrange("p (i h) -> p i h", h=2)
            nmj = pST.tile([P, NSEG], F32, tag="nmj")
            nc.vector.tensor_tensor(nmj[:], n0[:, :, 0], n0[:, :, 1], op=MIN)
            dm = pST.tile([P, 2 * NSEG], F32, tag="dm")
            d0 = dm[:].rearrange("p (i h) -> p i h", h=2)
            for h in range(2):
                nc.vector.tensor_sub(d0[:, :, h], n0[:, :, h], nmj[:])
            f = pST.tile([P, 2 * NSEG], F32, tag="f")
            nc.scalar.activation(f[:], dm[:], EXP, bias=0.0, scale=-1.0)
            lf = pST.tile([P, 2 * NSEG], F32, tag="lf")
            nc.vector.tensor_mul(lf[:], lsb[:], f[:])
            l0 = lf[:].rearrange("p (i h) -> p i h", h=2)
            lsum = pST.tile([P, NSEG], F32, tag="lsum")
            nc.vector.tensor_add(lsum[:], l0[:, :, 0], l0[:, :, 1])
            rcp = pST.tile([P, NSEG], F32, tag="rcp")
            nc.vector.reciprocal(rcp[:], lsum[:])
            g = pST.tile([P, 2 * NSEG], F32, tag="g")
            g0 = g[:].rearrange("p (i h) -> p i h", h=2)
            f0 = f[:].rearrange("p (i h) -> p i h", h=2)
            for h in range(2):
                nc.vector.tensor_mul(g0[:, :, h], f0[:, :, h], rcp[:])
            for i in range(NSEG):
                kj = seg * NSEG + i
                for h in range(2):
                    vs = pVS.tile([P, P], BF16, tag="vs")
                    if vs_act:
                        nc.scalar.activation(
                            vs[:], yv[:, ts(kj, P)], COPY,
                            scale=g[:, ds(i * 2 + h, 1)],
                        )
                    else:
                        nc.vector.tensor_scalar_mul(
                            vs[:], yv[:, ts(kj, P)], g[:, ds(i * 2 + h, 1)]
                        )
                    vss[kj, h] = vs

        def attn_seg(seg):
            for c in range(4):
                h = c // 2
                pa = psWK.tile([P, 512], F32, tag="wk")
                for i in range(NSEG):
                    kj = seg * NSEG + i
                    nc.tensor.matmul(
                        pa[:], vss[kj, h][:],
                        softs[kj, h][:, ds((c % 2) * 512, 512)],
                        start=(i == 0), stop=(i == NSEG - 1),
                    )
                if seg == 0:
                    nc.scalar.copy(acc[:, ds(c * 512, 512)], pa[:])
                else:
                    nc.vector.tensor_add(
                        acc[:, ds(c * 512, 512)], acc[:, ds(c * 512, 512)], pa[:]
                    )

        # Emission: seg-major so live softs stay within the pool (16+2);
        # seg0's vs on DVE (Act queue must not delay seg1's exps), seg1's
        # vs on Act (behind all of this pair's exps).  Filler slots cover
        # the PE bubble between a batch stage and its attn matmuls.
        for kj in range(NSEG):
            sc(kj)
        batch(0, vs_act=False)
        if filler is not None:
            filler(0)
        attn_seg(0)
        for kj in range(NSEG, 16):
            sc(kj)
        batch(1, vs_act=True)
        if filler is not None:
            filler(1)
        attn_seg(1)
        softs.clear()
        vss.clear()
        nc.sync.dma_start(out[ds(gp * P, P), :], acc[:])

    # Group pipeline: during C(G0), the 8 ec-chunks of Bv(G1) are fed in
    # as PE filler (2 chunks per pair, at the batch->attn bubbles).
    xtg0 = phase_x(0)
    phase_b_qk(0, xtg0)
    phase_b_v(0, xtg0)
    xtg1 = phase_x(1)
    phase_b_v_alloc(1)

    def mk_filler(pi):
        def filler(slot):
            phase_b_v_chunk(1, xtg1, pi * 2 + slot, copies_on_act=True)
        return filler

    for pi in range(GRP):
        phase_c(pi, pi, filler=mk_filler(pi))
    phase_b_qk(1, xtg1)
    for pi in range(GRP):
        phase_c(GRP + pi, pi)


def build(compile=True):
    key = ("nc", compile)
    if key in _cache:
        return _cache[key]
    nc = bacc.Bacc("TRN2", target_bir_lowering=False, debug=False)
    # x pre-transposed on host: [grp, pair, p(e-chunk col), kb, s]
    xl = nc.dram_tensor("xl", [NGRP, GRP, P, NJ, P], F32R, kind="ExternalInput").ap()
    wq = nc.dram_tensor("wq", [NJ, 2, P, 8, P], F32R, kind="ExternalInput").ap()
    wk = nc.dram_tensor("wk", [NJ, 2, P, 8, P], F32R, kind="ExternalInput").ap()
    wv = nc.dram_tensor("wv", [8, 4, P, 4, 256], F32R, kind="ExternalInput").ap()
    # out rows = (gp, d), cols = (class, r); host un-transposes.
    out = nc.dram_tensor("out", [NPAIR * P, 2048], F32, kind="ExternalOutput").ap()
    with tile.TileContext(nc) as tc:
        with ExitStack() as ctx:
            _emit(nc, tc, ctx, xl, wq, wk, wv, out)
    if compile:
        nc.compile()
    _cache[key] = nc
    return nc


def _prep_inputs(x, w_query, w_key, w_value):
    x = np.ascontiguousarray(np.asarray(x, np.float32))
    wq = np.asarray(w_query, np.float32)
    wk = np.asarray(w_key, np.float32)
    wv = np.asarray(w_value, np.float32)
    B, S, E = x.shape
    xf = x.reshape(B * S, E)
    # [j, half, p, kb8, q] ; 1/sqrt(Dh) folded into wq
    wq_t = np.ascontiguousarray(
        (wq * SCALE).reshape(NJ, P, NJ, P).transpose(2, 1, 0, 3)
        .reshape(NJ, P, 2, 8, P).transpose(0, 2, 1, 3, 4)
    )
    wk_t = np.ascontiguousarray(
        wk.reshape(NJ, P, NJ, P).transpose(2, 1, 0, 3)
        .reshape(NJ, P, 2, 8, P).transpose(0, 2, 1, 3, 4)
    )
    wv_t = np.ascontiguousarray(
        wv.reshape(NJ, P, 8, 256).transpose(2, 1, 0, 3)
        .reshape(8, P, 4, 4, 256).transpose(0, 2, 1, 3, 4)
    )
    rows = NPAIR * P
    in_maps = []
    for c in range(8):
        xc = xf[c * rows:(c + 1) * rows]
        # xtg[g][pi][p, kb, s] = xc[(g*4+pi)*128 + s, kb*128 + p]
        xt = np.ascontiguousarray(
            xc.reshape(NGRP, GRP, P, NJ, P).transpose(0, 1, 4, 3, 2)
        )
        in_maps.append(dict(xl=xt, wq=wq_t, wk=wk_t, wv=wv_t))
    return in_maps, (B, S, E)


def kernel(x, w_query, w_key, w_value, _want_trace=False):
    in_maps, (B, S, E) = _prep_inputs(x, w_query, w_key, w_value)
    nc = build()
    res = run_bass_kernel_spmd(nc, in_maps, core_ids=list(range(8)),
                               trace=_want_trace)
    # per-core out rows=(gp,d), cols=(class c, r); un-transpose to
    # rows=(gp,r), cols=(c,d)
    blocks = []
    for r in res.results:
        rc = r["out"].reshape(NPAIR, P, NJ, P)
        blocks.append(rc.transpose(0, 3, 2, 1).reshape(NPAIR * P, 2048))
    outf = np.concatenate(blocks, axis=0)
    if _want_trace:
        kernel.last_result = res
    return outf.reshape(B, S, E)


# revision 20
# speedup vs baseline: 1.3026x; 1.0776x over previous
"""MultiHeadAttention Trainium2 Bass kernel, 8-core SPMD — v2.

Problem: B=4, S=2048, E=2048, H=16, Dh=128; reshape-based (not transposed)
head split:  q = (x@Wq).reshape(B,H,S,Dh) etc., softmax over the QUERY axis,
out = attn.reshape(B,S,E).

Sharding: flattening (B,S) rows, row-block gp (128 rows) of x@W is exactly
head pair gp=(b,h).  Core c owns 8 consecutive pairs -> contiguous x rows
[1024c, 1024c+1024) and the same output rows.  No collectives.

v2 changes vs v1:
  - Q/K projections kept in SBUF (pair-major [d, pair, j, s] layout), no
    DRAM spill round-trip.
  - Softmax uses per-1024-half biases (own max via one fused
    tensor_tensor_reduce with scale=-1), flash-style: the exp(m_h - m_glob)
    correction folds into the per-half V stationaries.  PSUM score tiles
    free right after their exp -> PE never waits on a global-max join.
  - 1/sqrt(Dh) pre-folded into w_query on the host.
  - Attention accumulated in PSUM over 4-kj segments (acc add in SBUF).
  - Engine balance: maxes/recips on DVE, exps on Act, factor/vs scaling on
    GPSIMD (SBUF-only), copies split DVE/Act.
  - PSUM: scores 3x[128,1024] (6 banks) + shared work pool 2x[128,512].
"""

import numpy as np
from contextlib import ExitStack

import concourse.bass as bass
import concourse.tile as tile
from concourse import bacc, mybir
from concourse.bass import ds, ts
from concourse.bass_utils import run_bass_kernel_spmd
from concourse.masks import make_identity

F32 = mybir.dt.float32
F32R = mybir.dt.float32r
BF16 = mybir.dt.bfloat16
AX = mybir.AxisListType.X
EXP = mybir.ActivationFunctionType.Exp
COPY = mybir.ActivationFunctionType.Copy
MAX = mybir.AluOpType.max
MIN = mybir.AluOpType.min
MULT = mybir.AluOpType.mult
ADD = mybir.AluOpType.add

P = 128
NPAIR = 8          # (b,h) pairs per core
GRP = 4            # pairs per group (weights streamed once per group)
NGRP = NPAIR // GRP
NJ = 16            # 128-col blocks in E
NSEG = 8           # kj per attention accumulation segment
SCALE = 1.0 / np.sqrt(128.0)
FBIG = 3.0e38

_cache = {}


def _emit(nc, tc, ctx, xl, wq, wk, wv, idr, out):
    sb = ctx.enter_context
    # SBUF pools
    pXIN = sb(tc.tile_pool(name="pxin", bufs=1))     # x row-block f32r   8K
    pXT = sb(tc.tile_pool(name="pxt", bufs=1))       # XT group           32K
    pWQK = sb(tc.tile_pool(name="pwqk", bufs=3))     # w half-tiles       4K*3
    pWV = sb(tc.tile_pool(name="pwv", bufs=4))       # wv quarter-tiles   4K*4
    pQT = sb(tc.tile_pool(name="pqt", bufs=1))       # QT group           32K
    pKT = sb(tc.tile_pool(name="pkt", bufs=1))       # KT group           32K
    pYV = sb(tc.tile_pool(name="pyv", bufs=4))       # yv bf16 per pair   4K*4
    pSOFT = sb(tc.tile_pool(name="psoft", bufs=17))  # soft halves bf16   2K*17
    pVS = sb(tc.tile_pool(name="pvs", bufs=18))      # vs bf16 per half   .25K*18
    pACC = sb(tc.tile_pool(name="pacc", bufs=2))     # attnT acc f32      8K*2
    pST = sb(tc.tile_pool(name="pst", bufs=6))       # small stats        tiny
    pSCR = sb(tc.tile_pool(name="pscr", bufs=1))     # TTR scratch        tiny
    pCONST = sb(tc.tile_pool(name="pconst", bufs=1))
    # PSUM pools: 3*2 + 2*1 = 8 banks
    psSC = sb(tc.tile_pool(name="pssc", bufs=3, space="PSUM"))   # [128,1024]
    psWK = sb(tc.tile_pool(name="pswk", bufs=2, space="PSUM"))   # [128,512]

    ident = pCONST.tile([P, P], F32, tag="ident")
    make_identity(nc, ident[:])
    identr = pCONST.tile([P, P], F32R, tag="identr")
    nc.sync.dma_start(identr[:], idr)
    ident_r = identr[:]

    scr2 = pSCR.tile([P, 2], F32, tag="scr2")

    yv_tiles = {}
    qt_tiles = {}
    kt_tiles = {}

    def phase_a(grp):
        """Transpose the group's x blocks into XT [128, kb, pair, s] f32r."""
        xtg = pXT.tile([P, NJ, GRP, P], F32R, tag="xtg")
        for pi in range(GRP):
            gp = grp * GRP + pi
            xt = pXIN.tile([P, NJ * P], F32R, tag="xt")
            nc.sync.dma_start(xt[:], xl[ds(gp * P, P), :])
            for jj in range(4):
                pt = psWK.tile([P, 512], F32, tag="wk")
                for i in range(4):
                    j = jj * 4 + i
                    nc.tensor.transpose(
                        pt[:, ds(i * P, P)].bitcast(F32R), xt[:, ds(j * P, P)], ident_r
                    )
                nc.vector.tensor_copy(
                    xtg[:, ts(jj, 4), pi, :], pt[:].rearrange("p (a b) -> p a b", a=4)
                )
        return xtg

    def phase_b_qk(grp, xtg):
        """Q/K projections, j-major over the group's 4 pairs.

        Output layout: qt/kt [128(d), pair, j, s] f32r kept in SBUF."""
        qtg = pQT.tile([P, GRP, NJ, P], F32R, tag="qtg")
        ktg = pKT.tile([P, GRP, NJ, P], F32R, tag="ktg")
        for j in range(NJ):
            for wd, dstg in ((wq, qtg), (wk, ktg)):
                ps = psWK.tile([P, 512], F32, tag="wk")
                for h in range(2):
                    wt = pWQK.tile([P, 8, P], F32R, tag="wqk")
                    nc.sync.dma_start(wt[:], wd[j, h])
                    for kb8 in range(8):
                        kb = h * 8 + kb8
                        nc.tensor.matmul(
                            ps[:], wt[:, kb8], xtg[:, kb],
                            start=(kb == 0), stop=(kb == NJ - 1),
                        )
                nc.vector.tensor_copy(
                    dstg[:, :, j, :], ps[:].rearrange("p (g s) -> p g s", g=GRP)
                )
        for pi in range(GRP):
            gp = grp * GRP + pi
            qt_tiles[gp] = qtg
            kt_tiles[gp] = ktg

    def phase_b_v(grp, xtg):
        """V projections: yv[pair] = [128(s), 2048(e)] bf16 in SBUF."""
        for pi in range(GRP):
            yv_tiles[grp * GRP + pi] = pYV.tile(
                [P, NJ * P], BF16, tag="yv", name=f"yv{grp * GRP + pi}"
            )
        for ec in range(8):
            wvts = []
            for q in range(4):
                wvt = pWV.tile([P, 4, 256], F32R, tag="wv")
                nc.sync.dma_start(wvt[:], wv[ec, q])
                wvts.append(wvt)
            for pi in range(GRP):
                gp = grp * GRP + pi
                ps = psWK.tile([P, 512], F32, tag="wk")
                for kb in range(NJ):
                    nc.tensor.matmul(
                        ps[:, :256], xtg[:, kb, pi], wvts[kb // 4][:, kb % 4],
                        start=(kb == 0), stop=(kb == NJ - 1),
                    )
                nc.scalar.copy(yv_tiles[gp][:, ds(ec * 256, 256)], ps[:, :256])

    def phase_c(gp, pi):
        """Scores + per-half-bias softmax-over-q + attn + output, one pair."""
        qtg = qt_tiles.pop(gp)
        ktg = kt_tiles.pop(gp)
        yv = yv_tiles.pop(gp)
        acc = pACC.tile([P, NJ * P], F32, tag="acc")
        for seg in range(NJ // NSEG):
            softs = {}
            vss = {}
            for i in range(NSEG):
                kj = seg * NSEG + i
                kt_st = ktg[:, pi, kj, :]
                nm2 = pST.tile([P, 2], F32, tag="nm2")
                ls2 = pST.tile([P, 2], F32, tag="ls2")
                f2 = pST.tile([P, 2], F32, tag="f2")
                for h in range(2):
                    ps = psSC.tile([P, 1024], F32, tag="sc")
                    for c in range(2):
                        nc.tensor.matmul(
                            ps[:, ds(c * 512, 512)], kt_st,
                            qtg[:, pi, ds(h * 8 + c * 4, 4), :],
                            start=True, stop=True,
                        )
                    # nm2[:,h] = -max over the 1024 cols of this half
                    nc.vector.reduce_max(
                        nm2[:, ds(h, 1)], ps[:], axis=AX, negate=True
                    )
                    soft = pSOFT.tile([P, 1024], BF16, tag="soft")
                    nc.scalar.activation(
                        soft[:], ps[:], EXP,
                        bias=nm2[:, ds(h, 1)], scale=1.0,
                        accum_out=ls2[:, ds(h, 1)],
                    )
                    softs[kj, h] = soft
                # nmmin = min(nm0, nm1) = -m_glob
                nmmin = pST.tile([P, 1], F32, tag="nmmin")
                nc.vector.tensor_tensor(nmmin[:], nm2[:, :1], nm2[:, 1:], op=MIN)
                dm2 = pST.tile([P, 2], F32, tag="dm2")
                nc.vector.tensor_scalar_sub(dm2[:], nm2[:], nmmin[:])
                # f_h = exp(m_h - m_glob) = exp(-dm2)
                nc.scalar.activation(f2[:], dm2[:], EXP, bias=0.0, scale=-1.0)
                # L = sum_h Ls[h] * f[h];  rcp = 1/L;  g_h = f_h * rcp
                lf = pST.tile([P, 2], F32, tag="lf")
                nc.vector.tensor_mul(lf[:], ls2[:], f2[:])
                lsum = pST.tile([P, 1], F32, tag="lsum")
                nc.vector.reduce_sum(lsum[:], lf[:], axis=AX)
                rcp = pST.tile([P, 1], F32, tag="rcp")
                nc.vector.reciprocal(rcp[:], lsum[:])
                g2 = pST.tile([P, 2], F32, tag="g2")
                nc.vector.tensor_scalar_mul(g2[:], f2[:], rcp[:])
                for h in range(2):
                    vs = pVS.tile([P, P], BF16, tag="vs")
                    nc.vector.tensor_scalar_mul(
                        vs[:], yv[:, ts(kj, P)], g2[:, ds(h, 1)]
                    )
                    vss[kj, h] = vs
            # attn for this segment: 4 q-quarters x NSEG kjs, accumulated in
            # PSUM then added into acc.
            for c in range(4):
                h = c // 2
                pa = psWK.tile([P, 512], F32, tag="wk")
                for i in range(NSEG):
                    kj = seg * NSEG + i
                    nc.tensor.matmul(
                        pa[:], vss[kj, h][:],
                        softs[kj, h][:, ds((c % 2) * 512, 512)],
                        start=(i == 0), stop=(i == NSEG - 1),
                    )
                if seg == 0:
                    nc.scalar.copy(acc[:, ds(c * 512, 512)], pa[:])
                else:
                    nc.vector.tensor_add(
                        acc[:, ds(c * 512, 512)], acc[:, ds(c * 512, 512)], pa[:]
                    )
        # transpose acc (attnT) into output layout, in place per 512-chunk
        for jj in range(4):
            pt = psWK.tile([P, 512], F32, tag="wk")
            for i in range(4):
                cblk = jj * 4 + i
                nc.tensor.transpose(pt[:, ds(i * P, P)], acc[:, ds(cblk * P, P)], ident[:])
            nc.scalar.copy(acc[:, ds(jj * 512, 512)], pt[:])
        nc.sync.dma_start(out[ds(gp * P, P), :], acc[:])

    for grp in range(NGRP):
        xtg = phase_a(grp)
        phase_b_qk(grp, xtg)
        phase_b_v(grp, xtg)
        for pi in range(GRP):
            phase_c(grp * GRP + pi, pi)


def build(compile=True):
    key = ("nc", compile)
    if key in _cache:
        return _cache[key]
    nc = bacc.Bacc("TRN2", target_bir_lowering=False, debug=False)
    xl = nc.dram_tensor("xl", [NPAIR * P, 2048], F32R, kind="ExternalInput").ap()
    wq = nc.dram_tensor("wq", [NJ, 2, P, 8, P], F32R, kind="ExternalInput").ap()
    wk = nc.dram_tensor("wk", [NJ, 2, P, 8, P], F32R, kind="ExternalInput").ap()
    wv = nc.dram_tensor("wv", [8, 4, P, 4, 256], F32R, kind="ExternalInput").ap()
    idr = nc.dram_tensor("idr", [P, P], F32R, kind="ExternalInput").ap()
    out = nc.dram_tensor("out", [NPAIR * P, 2048], F32, kind="ExternalOutput").ap()
    with tile.TileContext(nc) as tc:
        with ExitStack() as ctx:
            _emit(nc, tc, ctx, xl, wq, wk, wv, idr, out)
    if compile:
        nc.compile()
    _cache[key] = nc
    return nc


def _prep_inputs(x, w_query, w_key, w_value):
    x = np.ascontiguousarray(np.asarray(x, np.float32))
    wq = np.asarray(w_query, np.float32)
    wk = np.asarray(w_key, np.float32)
    wv = np.asarray(w_value, np.float32)
    B, S, E = x.shape
    xf = x.reshape(B * S, E)
    # [j, half, p, kb8, q] ; 1/sqrt(Dh) folded into wq
    wq_t = np.ascontiguousarray(
        (wq * SCALE).reshape(NJ, P, NJ, P).transpose(2, 1, 0, 3)
        .reshape(NJ, P, 2, 8, P).transpose(0, 2, 1, 3, 4)
    )
    wk_t = np.ascontiguousarray(
        wk.reshape(NJ, P, NJ, P).transpose(2, 1, 0, 3)
        .reshape(NJ, P, 2, 8, P).transpose(0, 2, 1, 3, 4)
    )
    wv_t = np.ascontiguousarray(
        wv.reshape(NJ, P, 8, 256).transpose(2, 1, 0, 3)
        .reshape(8, P, 4, 4, 256).transpose(0, 2, 1, 3, 4)
    )
    eye = np.eye(P, dtype=np.float32)
    rows = NPAIR * P
    in_maps = [
        dict(xl=np.ascontiguousarray(xf[c * rows:(c + 1) * rows]),
             wq=wq_t, wk=wk_t, wv=wv_t, idr=eye)
        for c in range(8)
    ]
    return in_maps, (B, S, E)


def kernel(x, w_query, w_key, w_value, _want_trace=False):
    in_maps, (B, S, E) = _prep_inputs(x, w_query, w_key, w_value)
    nc = build()
    res = run_bass_kernel_spmd(nc, in_maps, core_ids=list(range(8)),
                               trace=_want_trace)
    outf = np.concatenate([r["out"] for r in res.results], axis=0)
    if _want_trace:
        kernel.last_result = res
    return outf.reshape(B, S, E)



# revision 26
# speedup vs baseline: 1.5170x; 1.1645x over previous
"""MultiHeadAttention Trainium2 Bass kernel, 8-core SPMD — v2.

Problem: B=4, S=2048, E=2048, H=16, Dh=128; reshape-based (not transposed)
head split:  q = (x@Wq).reshape(B,H,S,Dh) etc., softmax over the QUERY axis,
out = attn.reshape(B,S,E).

Sharding: flattening (B,S) rows, row-block gp (128 rows) of x@W is exactly
head pair gp=(b,h).  Core c owns 8 consecutive pairs -> contiguous x rows
[1024c, 1024c+1024) and the same output rows.  No collectives.

v2 changes vs v1:
  - Q/K projections kept in SBUF (pair-major [d, pair, j, s] layout), no
    DRAM spill round-trip.
  - Softmax uses per-1024-half biases (own max via one fused
    tensor_tensor_reduce with scale=-1), flash-style: the exp(m_h - m_glob)
    correction folds into the per-half V stationaries.  PSUM score tiles
    free right after their exp -> PE never waits on a global-max join.
  - 1/sqrt(Dh) pre-folded into w_query on the host.
  - Attention accumulated in PSUM over 4-kj segments (acc add in SBUF).
  - Engine balance: maxes/recips on DVE, exps on Act, factor/vs scaling on
    GPSIMD (SBUF-only), copies split DVE/Act.
  - PSUM: scores 3x[128,1024] (6 banks) + shared work pool 2x[128,512].
"""

import numpy as np
from contextlib import ExitStack

import concourse.bass as bass
import concourse.tile as tile
from concourse import bacc, mybir
from concourse.bass import ds, ts
from concourse.bass_utils import run_bass_kernel_spmd

F32 = mybir.dt.float32
F32R = mybir.dt.float32r
BF16 = mybir.dt.bfloat16
AX = mybir.AxisListType.X
EXP = mybir.ActivationFunctionType.Exp
COPY = mybir.ActivationFunctionType.Copy
MAX = mybir.AluOpType.max
MIN = mybir.AluOpType.min
MULT = mybir.AluOpType.mult
ADD = mybir.AluOpType.add

P = 128
NPAIR = 8          # (b,h) pairs per core
GRP = 4            # pairs per group (weights streamed once per group)
NGRP = NPAIR // GRP
NJ = 16            # 128-col blocks in E
NSEG = 8           # kj per attention accumulation segment
SCALE = 1.0 / np.sqrt(128.0)
FBIG = 3.0e38

_cache = {}


def _emit(nc, tc, ctx, xl, wq, wk, wv, out):
    sb = ctx.enter_context
    # SBUF pools
    pXT = sb(tc.tile_pool(name="pxt", bufs=1))       # XT group           32K
    pWQK = sb(tc.tile_pool(name="pwqk", bufs=4))     # w half-tiles       4K*4
    pWV = sb(tc.tile_pool(name="pwv", bufs=5))       # wv quarter-tiles   4K*5
    pQT = sb(tc.tile_pool(name="pqt", bufs=1))       # QT group           32K
    pKT = sb(tc.tile_pool(name="pkt", bufs=1))       # KT group           32K
    pYV = sb(tc.tile_pool(name="pyv", bufs=4))       # yv bf16 per pair   4K*4
    pSOFT = sb(tc.tile_pool(name="psoft", bufs=17))  # soft halves bf16   2K*17
    pVS = sb(tc.tile_pool(name="pvs", bufs=18))      # vs bf16 per half   .25K*18
    pACC = sb(tc.tile_pool(name="pacc", bufs=2))     # attnT acc f32      8K*2
    pST = sb(tc.tile_pool(name="pst", bufs=6))       # small stats        tiny
    pSCR = sb(tc.tile_pool(name="pscr", bufs=1))     # TTR scratch        tiny
    # PSUM pools: 3*2 + 2*1 = 8 banks
    psSC = sb(tc.tile_pool(name="pssc", bufs=3, space="PSUM"))   # [128,1024]
    psWK = sb(tc.tile_pool(name="pswk", bufs=2, space="PSUM"))   # [128,512]

    scr2 = pSCR.tile([P, 2], F32, tag="scr2")

    yv_tiles = {}
    qt_tiles = {}
    kt_tiles = {}

    def phase_a(grp):
        """DMA the group's pre-transposed x: [128(p), kb, pair, s] f32r."""
        xtg = pXT.tile([P, NJ, GRP, P], F32R, tag="xtg")
        for pi in range(GRP):
            nc.sync.dma_start(xtg[:, :, pi, :], xl[grp, pi])
        return xtg

    def phase_b_qk(grp, xtg):
        """Q/K projections, j-major over the group's 4 pairs.

        Output layout: qt/kt [128(d), pair, j, s] f32r kept in SBUF."""
        qtg = pQT.tile([P, GRP, NJ, P], F32R, tag="qtg")
        ktg = pKT.tile([P, GRP, NJ, P], F32R, tag="ktg")
        for j in range(NJ):
            for wd, dstg in ((wq, qtg), (wk, ktg)):
                ps = psWK.tile([P, 512], F32, tag="wk")
                for h in range(2):
                    wt = pWQK.tile([P, 8, P], F32R, tag="wqk")
                    nc.sync.dma_start(wt[:], wd[j, h])
                    for kb8 in range(8):
                        kb = h * 8 + kb8
                        nc.tensor.matmul(
                            ps[:], wt[:, kb8], xtg[:, kb],
                            start=(kb == 0), stop=(kb == NJ - 1),
                        )
                nc.vector.tensor_copy(
                    dstg[:, :, j, :], ps[:].rearrange("p (g s) -> p g s", g=GRP)
                )
        for pi in range(GRP):
            gp = grp * GRP + pi
            qt_tiles[gp] = qtg
            kt_tiles[gp] = ktg

    def phase_b_v(grp, xtg):
        """V projections: yv[pair] = [128(s), 2048(e)] bf16 in SBUF."""
        for pi in range(GRP):
            yv_tiles[grp * GRP + pi] = pYV.tile(
                [P, NJ * P], BF16, tag="yv", name=f"yv{grp * GRP + pi}"
            )
        for ec in range(8):
            wvts = []
            for q in range(4):
                wvt = pWV.tile([P, 4, 256], F32R, tag="wv")
                nc.sync.dma_start(wvt[:], wv[ec, q])
                wvts.append(wvt)
            for pi in range(GRP):
                gp = grp * GRP + pi
                ps = psWK.tile([P, 512], F32, tag="wk")
                for kb in range(NJ):
                    nc.tensor.matmul(
                        ps[:, :256], xtg[:, kb, pi], wvts[kb // 4][:, kb % 4],
                        start=(kb == 0), stop=(kb == NJ - 1),
                    )
                nc.vector.tensor_copy(yv_tiles[gp][:, ds(ec * 256, 256)], ps[:, :256])

    def phase_c(gp, pi):
        """Scores + per-half-bias softmax-over-q + attn + output, one pair."""
        qtg = qt_tiles.pop(gp)
        ktg = kt_tiles.pop(gp)
        yv = yv_tiles.pop(gp)
        acc = pACC.tile([P, NJ * P], F32, tag="acc")
        for seg in range(NJ // NSEG):
            softs = {}
            vss = {}
            for i in range(NSEG):
                kj = seg * NSEG + i
                kt_st = ktg[:, pi, kj, :]
                nm2 = pST.tile([P, 2], F32, tag="nm2")
                ls2 = pST.tile([P, 2], F32, tag="ls2")
                f2 = pST.tile([P, 2], F32, tag="f2")
                for h in range(2):
                    ps = psSC.tile([P, 1024], F32, tag="sc")
                    for c in range(2):
                        nc.tensor.matmul(
                            ps[:, ds(c * 512, 512)], kt_st,
                            qtg[:, pi, ds(h * 8 + c * 4, 4), :],
                            start=True, stop=True,
                        )
                    # nm2[:,h] = -max over the 1024 cols of this half
                    nc.vector.reduce_max(
                        nm2[:, ds(h, 1)], ps[:], axis=AX, negate=True
                    )
                    soft = pSOFT.tile([P, 1024], BF16, tag="soft")
                    nc.scalar.activation(
                        soft[:], ps[:], EXP,
                        bias=nm2[:, ds(h, 1)], scale=1.0,
                        accum_out=ls2[:, ds(h, 1)],
                    )
                    softs[kj, h] = soft
                # nmmin = min(nm0, nm1) = -m_glob
                nmmin = pST.tile([P, 1], F32, tag="nmmin")
                nc.vector.tensor_tensor(nmmin[:], nm2[:, :1], nm2[:, 1:], op=MIN)
                dm2 = pST.tile([P, 2], F32, tag="dm2")
                nc.vector.tensor_scalar_sub(dm2[:], nm2[:], nmmin[:])
                # f_h = exp(m_h - m_glob) = exp(-dm2)
                nc.scalar.activation(f2[:], dm2[:], EXP, bias=0.0, scale=-1.0)
                # L = sum_h Ls[h] * f[h];  rcp = 1/L;  g_h = f_h * rcp
                lf = pST.tile([P, 2], F32, tag="lf")
                nc.vector.tensor_mul(lf[:], ls2[:], f2[:])
                lsum = pST.tile([P, 1], F32, tag="lsum")
                nc.vector.reduce_sum(lsum[:], lf[:], axis=AX)
                rcp = pST.tile([P, 1], F32, tag="rcp")
                nc.vector.reciprocal(rcp[:], lsum[:])
                g2 = pST.tile([P, 2], F32, tag="g2")
                nc.vector.tensor_scalar_mul(g2[:], f2[:], rcp[:])
                for h in range(2):
                    vs = pVS.tile([P, P], BF16, tag="vs")
                    nc.vector.tensor_scalar_mul(
                        vs[:], yv[:, ts(kj, P)], g2[:, ds(h, 1)]
                    )
                    vss[kj, h] = vs
            # attn for this segment: 4 q-quarters x NSEG kjs, accumulated in
            # PSUM then added into acc.
            for c in range(4):
                h = c // 2
                pa = psWK.tile([P, 512], F32, tag="wk")
                for i in range(NSEG):
                    kj = seg * NSEG + i
                    nc.tensor.matmul(
                        pa[:], vss[kj, h][:],
                        softs[kj, h][:, ds((c % 2) * 512, 512)],
                        start=(i == 0), stop=(i == NSEG - 1),
                    )
                if seg == 0:
                    nc.scalar.copy(acc[:, ds(c * 512, 512)], pa[:])
                else:
                    nc.vector.tensor_add(
                        acc[:, ds(c * 512, 512)], acc[:, ds(c * 512, 512)], pa[:]
                    )
        # acc stays in [d, (class, r)] layout; the host un-transposes.
        nc.sync.dma_start(out[ds(gp * P, P), :], acc[:])

    for grp in range(NGRP):
        xtg = phase_a(grp)
        phase_b_qk(grp, xtg)
        phase_b_v(grp, xtg)
        for pi in range(GRP):
            phase_c(grp * GRP + pi, pi)


def build(compile=True):
    key = ("nc", compile)
    if key in _cache:
        return _cache[key]
    nc = bacc.Bacc("TRN2", target_bir_lowering=False, debug=False)
    # x pre-transposed on host: [grp, pair, p(e-chunk col), kb, s]
    xl = nc.dram_tensor("xl", [NGRP, GRP, P, NJ, P], F32R, kind="ExternalInput").ap()
    wq = nc.dram_tensor("wq", [NJ, 2, P, 8, P], F32R, kind="ExternalInput").ap()
    wk = nc.dram_tensor("wk", [NJ, 2, P, 8, P], F32R, kind="ExternalInput").ap()
    wv = nc.dram_tensor("wv", [8, 4, P, 4, 256], F32R, kind="ExternalInput").ap()
    out = nc.dram_tensor("out", [NPAIR * P, 2048], F32, kind="ExternalOutput").ap()
    with tile.TileContext(nc) as tc:
        with ExitStack() as ctx:
            _emit(nc, tc, ctx, xl, wq, wk, wv, out)
    if compile:
        nc.compile()
    _cache[key] = nc
    return nc


def _prep_inputs(x, w_query, w_key, w_value):
    x = np.ascontiguousarray(np.asarray(x, np.float32))
    wq = np.asarray(w_query, np.float32)
    wk = np.asarray(w_key, np.float32)
    wv = np.asarray(w_value, np.float32)
    B, S, E = x.shape
    xf = x.reshape(B * S, E)
    # [j, half, p, kb8, q] ; 1/sqrt(Dh) folded into wq
    wq_t = np.ascontiguousarray(
        (wq * SCALE).reshape(NJ, P, NJ, P).transpose(2, 1, 0, 3)
        .reshape(NJ, P, 2, 8, P).transpose(0, 2, 1, 3, 4)
    )
    wk_t = np.ascontiguousarray(
        wk.reshape(NJ, P, NJ, P).transpose(2, 1, 0, 3)
        .reshape(NJ, P, 2, 8, P).transpose(0, 2, 1, 3, 4)
    )
    wv_t = np.ascontiguousarray(
        wv.reshape(NJ, P, 8, 256).transpose(2, 1, 0, 3)
        .reshape(8, P, 4, 4, 256).transpose(0, 2, 1, 3, 4)
    )
    rows = NPAIR * P
    in_maps = []
    for c in range(8):
        xc = xf[c * rows:(c + 1) * rows]
        # xtg[g][pi][p, kb, s] = xc[(g*4+pi)*128 + s, kb*128 + p]
        xt = np.ascontiguousarray(
            xc.reshape(NGRP, GRP, P, NJ, P).transpose(0, 1, 4, 3, 2)
        )
        in_maps.append(dict(xl=xt, wq=wq_t, wk=wk_t, wv=wv_t))
    return in_maps, (B, S, E)


def kernel(x, w_query, w_key, w_value, _want_trace=False):
    in_maps, (B, S, E) = _prep_inputs(x, w_query, w_key, w_value)
    nc = build()
    res = run_bass_kernel_spmd(nc, in_maps, core_ids=list(range(8)),
                               trace=_want_trace)
    # per-core out rows=(gp,d), cols=(class c, r); un-transpose to
    # rows=(gp,r), cols=(c,d)
    blocks = []
    for r in res.results:
        rc = r["out"].reshape(NPAIR, P, NJ, P)
        blocks.append(rc.transpose(0, 3, 2, 1).reshape(NPAIR * P, 2048))
    outf = np.concatenate(blocks, axis=0)
    if _want_trace:
        kernel.last_result = res
    return outf.reshape(B, S, E)

